# revision 7
# baseline (speedup 1.0000x reference)
"""Trainium2 Bass kernel for MultiHeadSelfAttention with ALiBi + adj bias.

Reference computation (B=2, L=2048, H=1024, NH=16, HS=64):
    xp = x @ weights + in_bias                  # [b, l, 3h], per-head interleaved qkv
    q, k, v per head; att = q k^T / 8 + alibi + gamma*adj; softmax
    out = (att @ v) @ out_w + out_bias

Sharding: 8 cores = 2 batches x 4 slot-groups. Core (b, g) owns four head
"slots": plain0=8+2g, plain1=9+2g (no ALiBi), heavy=4+g (small slopes,
full attention), light=g (large slopes 2^-1..2^-4). Light-slot attention
is banded: weights with slope*|i-j| > ~24 are < e^-24 of the row max, so
only an 8-j-tile window around the diagonal is processed.

TimelineSim cost facts this design exploits: matmul costs out_free_cols
x 1 cyc at bf16, 0.5 at fp8e4 DoubleRow (planes sum two KxMxN products);
matmul weight loads are free, so att@V with the E tile stationary costs
65 cols instead of 512; ACT costs ~0.83ns/col + ~400ns/instruction, so
exp runs 1024-wide; fp8 anywhere on the logit/value path fails the 2e-2
gate unless split hi/lo (verified numerically), so:
  - QKV projection: fp8 DoubleRow 3-product (xh@wh + xh@wl + xl@wh) over
    k-tile-pair planes, weights prescaled x16 to clear fp8 subnormals.
    Q evacuates to fp8 (x 1/8/16), K to an (hi, lo) fp8 split via a -16I
    follow-up matmul, V to bf16 token-major (+ ones column) and to fp8
    j-pair tiles.
  - S^T[j,i]: DoubleRow (Kh,Kl) x broadcast-Q8; gamma*adj added by fp8
    DoubleRow identity matmuls with (hi, lo) planes (~12-bit); slope*dist
    likewise with planes (16s*I, s*I) against (dist/16, residual) - EXACT.
  - Plain slots: E8 = exp(S-4) straight to fp8; att@V is DoubleRow with
    j-pair planes (V8 stationary), out [65, i] d-major, row 64 = denom.
  - Alibi slots: E bf16, att@V flipped (E stationary, V moving, 65-col),
    [i, d] result PE-transposed back per chunk.
  - The output projection runs inside the chunk loop (bf16), so its PE
    work fills ACT-bound gaps and there is no serial tail.
Host sums the 4 bf16 y^T partials per batch, transposes, adds out_bias.
gamma rides fp8 identity weights: exact for gamma=1 / powers of two.
"""

import numpy as np
import ml_dtypes
from contextlib import ExitStack

import concourse.tile as tile
from concourse import bacc, mybir
from concourse import bass_utils

F32 = mybir.dt.float32
F32R = mybir.dt.float32r
BF16 = mybir.dt.bfloat16
F8 = mybir.dt.float8e4
AF = mybir.ActivationFunctionType
DR = mybir.MatmulPerfMode.DoubleRow
NPF8 = ml_dtypes.float8_e4m3
NPBF16 = ml_dtypes.bfloat16

B, L, H, NH = 2, 2048, 1024, 16
HS = 64
P = 128            # partition tile
IC = 512           # i-chunk width
NC = L // IC       # 4 i-chunks
NJ = L // P        # 16 j tiles
KT = H // P        # 8 contraction tiles over hidden
KP = KT // 2       # 4 k-tile pairs (DoubleRow planes)
SCALE = 0.125      # 1/sqrt(HS)
WS = 16.0          # weight prescale (fp8 subnormal clearance)
E8SHIFT = -4.0     # exp shift keeps fp8 E8 = exp(S-4) under e4m3 max 240
LWIN = 8           # light-slot j-tile window per 256-col sub-chunk

RUN_KWARGS: dict = {}
_cache: dict = {}

# Schraudolph fast-exp constants for DVE int16-bitcast bf16 exp:
# bf16 bits = trunc(A16*S + B16); bitcast(bits) ~ exp(S). Valid for
# S > -88 (bits stay positive); alibi slots satisfy this by windowing
# (light) or slope <= 1/32 (heavy).
A16 = 128.0 / float(np.log(2.0))
B16 = 127.0 * 128.0 + 0.5


def _build_program(with_qk_bias=False):
    nc = bacc.Bacc("TRN2", target_bir_lowering=False, debug=False,
                   enable_asserts=False, num_devices=8)

    xh = nc.dram_tensor("xh", [P, KP, 2, L], F8, kind="ExternalInput").ap()
    xl = nc.dram_tensor("xl", [P, KP, 2, L], F8, kind="ExternalInput").ap()
    wqh = nc.dram_tensor("wqh", [P, KP, 2, 512], F8, kind="ExternalInput").ap()
    wql = nc.dram_tensor("wql", [P, KP, 2, 512], F8, kind="ExternalInput").ap()
    wvh = nc.dram_tensor("wvh", [P, KP, 2, 260], F8, kind="ExternalInput").ap()
    wvl = nc.dram_tensor("wvl", [P, KP, 2, 260], F8, kind="ExternalInput").ap()
    vb = nc.dram_tensor("vb", [1, 260], BF16, kind="ExternalInput").ap()
    ones = nc.dram_tensor("ones", [1, IC], BF16, kind="ExternalInput").ap()
    qkb = (nc.dram_tensor("qkb", [1, 512], BF16, kind="ExternalInput").ap()
           if with_qk_bias else None)
    adjp = nc.dram_tensor("adjp", [P, NJ, 2, L], F8, kind="ExternalInput").ap()
    distp = nc.dram_tensor("distp", [P, NJ, 2, L], F8,
                           kind="ExternalInput").ap()
    idg = nc.dram_tensor("idg", [P, 2, 512], F8, kind="ExternalInput").ap()
    idsl = nc.dram_tensor("idsl", [P, 2, 256], F8, kind="ExternalInput").ap()
    identT = nc.dram_tensor("identT", [P, P], BF16, kind="ExternalInput").ap()
    idn = nc.dram_tensor("idn", [P, P], F8, kind="ExternalInput").ap()
    ow = nc.dram_tensor("ow", [P, 2, H], BF16, kind="ExternalInput").ap()
    yT = nc.dram_tensor("yT", [H, L], BF16, kind="ExternalOutput").ap()
    DEBUG = bool(_cache.get("debug"))
    if DEBUG:
        dbg_attnT0 = nc.dram_tensor("dbg_attnT0", [P, L], BF16, kind="ExternalOutput").ap()
        dbg_attnT1 = nc.dram_tensor("dbg_attnT1", [P, L], BF16, kind="ExternalOutput").ap()
        dbg_q80 = nc.dram_tensor("dbg_q80", [P, L], F8, kind="ExternalOutput").ap()
        dbg_q81 = nc.dram_tensor("dbg_q81", [P, L], F8, kind="ExternalOutput").ap()
        dbg_khl0 = nc.dram_tensor("dbg_khl0", [P, 2, L], F8, kind="ExternalOutput").ap()
        dbg_khl1 = nc.dram_tensor("dbg_khl1", [P, 2, L], F8, kind="ExternalOutput").ap()
        dbg_v0 = nc.dram_tensor("dbg_v0", [P, 260], BF16, kind="ExternalOutput").ap()
        dbg_v800 = nc.dram_tensor("dbg_v800", [P, 2, P], F8, kind="ExternalOutput").ap()
        dbg_atfh = nc.dram_tensor("dbg_atfh", [P, 4, 65], F32, kind="ExternalOutput").ap()
        dbg_attf0 = nc.dram_tensor("dbg_attf0", [P, 4, HS], BF16, kind="ExternalOutput").ap()
        dbg_et0 = nc.dram_tensor("dbg_et0", [P, 2, IC], BF16, kind="ExternalOutput").ap()

    with tile.TileContext(nc) as tc, ExitStack() as ctx:
        # ---- long-lived tensors ----
        persist = ctx.enter_context(tc.tile_pool(name="persist", bufs=1))
        # Q col-major fp8 (1/8 folded): tile0 rows = Q(p0 0:64, p1 64:128),
        # tile1 = Q(hv, lt). K as (hi, lo) fp8 plane pairs, same row split.
        q8_sb = [persist.tile([P, L], F8, tag=f"q8{m}", name=f"q8{m}")
                 for m in range(2)]
        khl_sb = [persist.tile([P, 2, L], F8, tag=f"khl{m}", name=f"khl{m}")
                  for m in range(2)]
        # V token-major bf16 [tok, 4*65] (65th col of each slot = 1.0)
        v_sb = [persist.tile([P, 260], BF16, tag=f"v{t}", name=f"v{t}")
                for t in range(NJ)]
        # V fp8 j-pair tiles for plain-slot DoubleRow att@V. Walrus requires
        # DoubleRow stationary planes of exactly 128 columns, so each slot's
        # 65 V columns (65th = ones) are zero-padded to 128.
        v8_sb = [[persist.tile([P, 2, P], F8, tag=f"v8_{s}_{t}",
                               name=f"v8_{s}_{t}") for t in range(NJ // 2)]
                 for s in range(2)]
        # normalized attention, d-major: tile0 = plain0+plain1, tile1 = hv+lt
        attnT = [persist.tile([P, L], BF16, tag=f"attnT{m}", name=f"attnT{m}")
                 for m in range(2)]
        # alibi slots' flipped attention [i, d] for one chunk
        attn_f = [persist.tile([P, 4, HS], BF16, tag=f"attf{m}",
                               name=f"attf{m}") for m in range(2)]
        ones_sb = persist.tile([1, IC], BF16, tag="ones")
        nc.sync.dma_start(ones_sb[:], ones)
        idg_sb = persist.tile([P, 2, 512], F8, tag="idg")
        nc.sync.dma_start(idg_sb[:], idg)
        idsl_sb = persist.tile([P, 2, 256], F8, tag="idsl")
        nc.sync.dma_start(idsl_sb[:], idsl)
        idT_sb = persist.tile([P, P], BF16, tag="identT")
        nc.sync.dma_start(idT_sb[:], identT)
        idn_sb = persist.tile([P, P], F8, tag="idn")
        nc.sync.dma_start(idn_sb[:], idn)
        vb_sb = persist.tile([1, 260], BF16, tag="vb")
        nc.sync.dma_start(vb_sb[:], vb)
        ow_sb = persist.tile([P, 2, H], BF16, tag="ow")
        nc.sync.dma_start(ow_sb[:], ow)
        if with_qk_bias:
            qkb_sb = persist.tile([1, 512], BF16, tag="qkb")
            nc.sync.dma_start(qkb_sb[:], qkb)
        mb_sb = persist.tile([P, 1], F32, tag="mb")
        nc.vector.memset(mb_sb[:], E8SHIFT)
        for s in range(2):
            for t in range(NJ // 2):
                nc.vector.memset(v8_sb[s][t][:], 0.0)

        # ========== Phase A: QKV projection (fp8 3-product) ==========
        bc_pool = ctx.enter_context(tc.tile_pool(name="bias_ch", bufs=3))
        adj_pre = {}
        dist_pre = {}
        with tc.tile_pool(name="xw", bufs=1) as xw_pool, \
             tc.tile_pool(name="psA", bufs=1, space="PSUM") as psA:
            wvh_sb = xw_pool.tile([P, KP, 2, 260], F8, tag="wvh")
            nc.sync.dma_start(wvh_sb[:], wvh)
            wvl_sb = xw_pool.tile([P, KP, 2, 260], F8, tag="wvl")
            nc.sync.dma_start(wvl_sb[:], wvl)
            xh_sb = xw_pool.tile([P, KP, 2, L], F8, tag="xh")
            xl_sb = xw_pool.tile([P, KP, 2, L], F8, tag="xl")
            nc.sync.dma_start(xh_sb[:, :, :, 0:256], xh[:, :, :, 0:256])
            nc.sync.dma_start(xl_sb[:, :, :, 0:256], xl[:, :, :, 0:256])
            nc.sync.dma_start(xh_sb[:, :, :, 256:IC], xh[:, :, :, 256:IC])
            nc.sync.dma_start(xl_sb[:, :, :, 256:IC], xl[:, :, :, 256:IC])
            wqh_sb = xw_pool.tile([P, KP, 2, 512], F8, tag="wqh")
            nc.sync.dma_start(wqh_sb[:], wqh)
            wql_sb = xw_pool.tile([P, KP, 2, 512], F8, tag="wql")
            nc.sync.dma_start(wql_sb[:], wql)
            for cc in range(1, NC):
                csl = slice(cc * IC, (cc + 1) * IC)
                nc.sync.dma_start(xh_sb[:, :, :, csl], xh[:, :, :, csl])
                nc.sync.dma_start(xl_sb[:, :, :, csl], xl[:, :, :, csl])
            for c0 in range(2):
                csl = slice(c0 * IC, (c0 + 1) * IC)
                a_t = bc_pool.tile([P, NJ, 2, IC], F8, tag="adj",
                                   name=f"adj_pre{c0}")
                nc.sync.dma_start(a_t[:], adjp[:, :, :, csl])
                adj_pre[c0] = a_t
                d_t = bc_pool.tile([P, NJ, 2, IC], F8, tag="dist",
                                   name=f"dist_pre{c0}")
                nc.sync.dma_start(d_t[:], distp[:, :, :, csl])
                dist_pre[c0] = d_t

            # V token-major (psum = 16*v; evacuations scale by 1/16).
            # Two token tiles pair into one [P, 2, 260] psum (one bank
            # start) so each evacuation instruction covers both.
            for tp in range(NJ // 2):
                ps = psA.tile([P, 2, 512], F32, tag="vp", bufs=2)
                for tt in range(2):
                    t = 2 * tp + tt
                    tsl = slice(t * P, (t + 1) * P)
                    nc.tensor.matmul(ps[:, tt, 0:260], ones_sb[:, 0:P],
                                     vb_sb[:], start=True, stop=False,
                                     skip_group_check=True)
                    for kp in range(KP):
                        for pi, (sta, mov) in enumerate(
                                ((xh_sb, wvh_sb), (xh_sb, wvl_sb),
                                 (xl_sb, wvh_sb))):
                            for hf in range(2):
                                nc.tensor.matmul(
                                    ps[:, tt, hf * 130:(hf + 1) * 130],
                                    sta[:, kp, :, tsl],
                                    mov[:, kp, :, hf * 130:(hf + 1) * 130],
                                    start=False,
                                    stop=(kp == KP - 1 and pi == 2
                                          and hf == 1 and tt == 1),
                                    perf_mode=DR, skip_group_check=True)
                for tt in range(2):
                    nc.vector.tensor_scalar_mul(v_sb[2 * tp + tt][:],
                                                ps[:, tt, 0:260], 1.0 / WS)
                for s in range(2):
                    nc.scalar.activation(
                        v8_sb[s][tp][:, :, 0:65],
                        ps[:, :, s * 65:(s + 1) * 65], AF.Copy,
                        scale=1.0 / WS)

            # Q^T / K^T col-major (psum = 16*q or 16*k). Two 256-col
            # DoubleRow outputs pair into one [P, 2, 256] psum so the
            # ScalarE evacuations run 512 wide; the pair shares one
            # start=True (ZERO_REGION covers the bank).
            for m in (1, 3, 0, 2):
                for cc in range(NC):
                    csl = slice(cc * IC, (cc + 1) * IC)
                    ps = psA.tile([P, 2, 256], F32, tag="qkp", bufs=4)
                    for half in range(2):
                        hsl = slice(cc * IC + half * 256,
                                    cc * IC + (half + 1) * 256)
                        if with_qk_bias:
                            nc.tensor.matmul(ps[:, half, :],
                                             qkb_sb[:, m * P:(m + 1) * P],
                                             ones_sb[:, 0:256],
                                             start=(half == 0), stop=False,
                                             skip_group_check=True)
                        for kp in range(KP):
                            for pi, (sta, mov) in enumerate(
                                    ((wqh_sb, xh_sb), (wql_sb, xh_sb),
                                     (wqh_sb, xl_sb))):
                                nc.tensor.matmul(
                                    ps[:, half, :],
                                    sta[:, kp, :, m * P:(m + 1) * P],
                                    mov[:, kp, :, hsl],
                                    start=(half == 0 and kp == 0 and pi == 0
                                           and not with_qk_bias),
                                    stop=(kp == KP - 1 and pi == 2
                                          and m < 2 and half == 1),
                                    perf_mode=DR, skip_group_check=True)
                    if m < 2:
                        nc.scalar.activation(q8_sb[m][:, csl], ps[:], AF.Copy,
                                             scale=SCALE / WS)
                    else:
                        kh = khl_sb[m - 2]
                        nc.scalar.activation(kh[:, 0, csl], ps[:], AF.Copy,
                                             scale=1.0 / WS)
                        nc.tensor.matmul(ps[:], idn_sb[:],
                                         kh[:, 0, csl], start=False,
                                         stop=True, skip_group_check=True)
                        nc.scalar.activation(kh[:, 1, csl], ps[:], AF.Copy,
                                             scale=1.0 / WS)

        # ========== Phase B: attention + fused output projection ==========
        with tc.tile_pool(name="e8_pool", bufs=6) as e8_pool, \
             tc.tile_pool(name="e_pool", bufs=6) as e_pool, \
             tc.tile_pool(name="r_pool", bufs=4) as r_pool, \
             tc.tile_pool(name="y_pool", bufs=4) as y_pool, \
             tc.tile_pool(name="psS", bufs=2, space="PSUM") as psS, \
             tc.tile_pool(name="psAc", bufs=2, space="PSUM") as psAc, \
             tc.tile_pool(name="psF", bufs=1, space="PSUM") as psF, \
             tc.tile_pool(name="psM", bufs=2, space="PSUM") as psM:
            for c in range(NC):
                csl = slice(c * IC, (c + 1) * IC)
                if c in adj_pre:
                    adj_ch = adj_pre[c]
                    dist_ch = dist_pre[c]
                else:
                    adj_ch = bc_pool.tile([P, NJ, 2, IC], F8, tag="adj",
                                          name=f"adj{c}")
                    nc.sync.dma_start(adj_ch[:], adjp[:, :, :, csl])
                    dist_ch = bc_pool.tile([P, NJ, 2, IC], F8, tag="dist",
                                           name=f"dist{c}")
                    nc.sync.dma_start(dist_ch[:], distp[:, :, :, csl])

                # ---- plain slots: E8/V8 DoubleRow att@V, d-major out ----
                for s in range(2):
                    hp = slice(s * HS, (s + 1) * HS)
                    atp = psAc.tile([P, IC], F32, tag="atp", name="atp", bufs=1)
                    for jp in range(NJ // 2):
                        e8p = e8_pool.tile([P, 2, IC], F8, tag="e8")
                        spp = psS.tile([P, 2, IC], F32, tag="sp", name="spp")
                        for jj in range(2):
                            j = 2 * jp + jj
                            for h2 in range(2):
                                h2s = slice(h2 * 256, (h2 + 1) * 256)
                                qmv = q8_sb[0][hp, c * IC + h2 * 256:
                                               c * IC + (h2 + 1) * 256]
                                nc.tensor.matmul(
                                    spp[:, jj, h2s],
                                    khl_sb[0][hp, :, j * P:(j + 1) * P],
                                    qmv.unsqueeze(1).broadcast_to(
                                        [HS, 2, 256]),
                                    start=True, stop=False, perf_mode=DR)
                                nc.tensor.matmul(
                                    spp[:, jj, h2s],
                                    idg_sb[:, :, s * P:(s + 1) * P],
                                    adj_ch[:, j, :, h2s], start=False,
                                    stop=True, perf_mode=DR)
                        nc.scalar.activation(e8p[:], spp[:], AF.Exp,
                                             bias=mb_sb[:])
                        for h2 in range(2):
                            h2s = slice(h2 * 256, (h2 + 1) * 256)
                            nc.tensor.matmul(
                                atp[:, h2s], v8_sb[s][jp][:],
                                e8p[:, :, h2s],
                                start=(jp == 0 and h2 == 0),
                                stop=(jp == NJ // 2 - 1), perf_mode=DR,
                                skip_group_check=True)
                    rec = r_pool.tile([1, IC], F32R, tag="rec")
                    with nc.allow_low_precision(reason="softmax denom recip"):
                        nc.vector.reciprocal(rec[:], atp[64:65, :])
                    rbs = r_pool.tile([HS, IC], F32, tag="rbs")
                    nc.gpsimd.partition_broadcast(rbs[:], rec[:].bitcast(F32))
                    nc.vector.tensor_mul(attnT[0][hp, csl], atp[0:HS, :],
                                         rbs[:])

                # ---- heavy slot: full attention, flipped att@V ----
                atf_h = psF.tile([P, 4, 65], F32, tag="atf", name="atf_h")
                for jp in range(NJ // 2):
                    spp = psS.tile([P, 2, IC], F32, tag="sp", name="sph")
                    for jj in range(2):
                        j = 2 * jp + jj
                        for h2 in range(2):
                            h2s = slice(h2 * 256, (h2 + 1) * 256)
                            qmv = q8_sb[1][0:HS, c * IC + h2 * 256:
                                           c * IC + (h2 + 1) * 256]
                            nc.tensor.matmul(
                                spp[:, jj, h2s],
                                khl_sb[1][0:HS, :, j * P:(j + 1) * P],
                                qmv.unsqueeze(1).broadcast_to([HS, 2, 256]),
                                start=True, stop=False, perf_mode=DR)
                            nc.tensor.matmul(spp[:, jj, h2s],
                                             idg_sb[:, :, 2 * P:3 * P],
                                             adj_ch[:, j, :, h2s],
                                             start=False, stop=False,
                                             perf_mode=DR)
                            nc.tensor.matmul(spp[:, jj, h2s],
                                             idsl_sb[:, :, 0:P],
                                             dist_ch[:, j, :, h2s],
                                             start=False, stop=True,
                                             perf_mode=DR)
                    et = e_pool.tile([P, 2, IC], BF16, tag="et")
                    nc.vector.tensor_scalar(et[:].bitcast(mybir.dt.int16),
                                            spp[:], A16, B16,
                                            mybir.AluOpType.mult,
                                            mybir.AluOpType.add)
                    if DEBUG and c == 0 and jp == 0:
                        nc.sync.dma_start(dbg_et0, et[:])
                    for jj in range(2):
                        for blk in range(4):
                            nc.tensor.matmul(
                                atf_h[:, blk, :],
                                et[:, jj, blk * P:(blk + 1) * P],
                                v_sb[2 * jp + jj][:, 130:195],
                                start=(jp == 0 and jj == 0 and blk == 0),
                                stop=(jp == NJ // 2 - 1 and jj == 1),
                                skip_group_check=True)

                rec4 = r_pool.tile([P, 4], F32, tag="rec4")
                with nc.allow_low_precision(reason="softmax denom recip"):
                    nc.vector.reciprocal(rec4[:], atf_h[:, :, 64])
                if DEBUG and c == 0:
                    dbh = r_pool.tile([P, 4, 65], F32, tag="dbh")
                    nc.scalar.activation(dbh[:], atf_h[:], AF.Copy)
                    nc.sync.dma_start(dbg_atfh, dbh[:])
                for blk in range(4):
                    nc.vector.tensor_scalar_mul(
                        attn_f[0][:, blk, :], atf_h[:, blk, 0:HS],
                        rec4[:, blk:blk + 1])
                if DEBUG and c == 0:
                    nc.sync.dma_start(dbg_attf0, attn_f[0][:])

                # ---- light slot: windowed attention, flipped att@V ----
                atf_l = psF.tile([P, 4, 65], F32, tag="atf", name="atf_l")
                for sub in range(2):
                    jw = min(max(4 * c + 2 * sub - 3, 0), NJ - LWIN)
                    ssl = slice(c * IC + sub * 256, c * IC + sub * 256 + 256)
                    bsl = slice(sub * 256, sub * 256 + 256)
                    qmv = q8_sb[1][HS:P, ssl].unsqueeze(1)
                    for jq in range(2):
                        spq = psS.tile([P, 4, 256], F32, tag="sp", name="spq")
                        for j4 in range(4):
                            j = jw + 4 * jq + j4
                            nc.tensor.matmul(
                                spq[:, j4, :],
                                khl_sb[1][HS:P, :, j * P:(j + 1) * P],
                                qmv.broadcast_to([HS, 2, 256]),
                                start=True, stop=False, perf_mode=DR)
                            nc.tensor.matmul(spq[:, j4, :],
                                             idg_sb[:, :, 3 * P:4 * P],
                                             adj_ch[:, j, :, bsl],
                                             start=False, stop=False,
                                             perf_mode=DR)
                            nc.tensor.matmul(spq[:, j4, :],
                                             idsl_sb[:, :, P:2 * P],
                                             dist_ch[:, j, :, bsl],
                                             start=False, stop=True,
                                             perf_mode=DR)
                        et = e_pool.tile([P, 4, 256], BF16, tag="etl")
                        nc.scalar.activation(et[:], spq[:], AF.Exp)
                        for j4 in range(4):
                            for blk in range(2):
                                nc.tensor.matmul(
                                    atf_l[:, sub * 2 + blk, :],
                                    et[:, j4, blk * P:(blk + 1) * P],
                                    v_sb[jw + 4 * jq + j4][:, 195:260],
                                    start=(sub == 0 and jq == 0 and j4 == 0
                                           and blk == 0),
                                    stop=(jq == 1 and j4 == 3),
                                    skip_group_check=True)


                rec4 = r_pool.tile([P, 4], F32, tag="rec4")
                with nc.allow_low_precision(reason="softmax denom recip"):
                    nc.vector.reciprocal(rec4[:], atf_l[:, :, 64])
                for blk in range(4):
                    nc.vector.tensor_scalar_mul(
                        attn_f[1][:, blk, :], atf_l[:, blk, 0:HS],
                        rec4[:, blk:blk + 1])

                # transpose alibi attention [i, d] -> d-major attnT[1]
                for fi in range(2):
                    rp = slice(fi * HS, (fi + 1) * HS)
                    pst = psM.tile([HS, 4, P], BF16, tag="misc", name="pst")
                    for k4 in range(4):
                        nc.tensor.matmul(pst[:, k4, :], attn_f[fi][:, k4, :],
                                         idT_sb[:], start=True, stop=True,
                                         is_transpose=True)
                    nc.vector.tensor_copy(attnT[1][rp, csl], pst[:])

                # fused output projection for this chunk: evacuations
                # batch into [P, 8, 256] staging, one DMA per 256-col half
                for hf in range(2):
                    ysl = slice(c * IC + hf * 256, c * IC + (hf + 1) * 256)
                    yt = y_pool.tile([P, 8, 256], BF16, tag="yt")
                    for m in range(H // P):
                        ps = psM.tile([P, 256], F32, tag="misc", name="yp")
                        nc.tensor.matmul(ps[:],
                                         ow_sb[:, 0, m * P:(m + 1) * P],
                                         attnT[0][:, ysl],
                                         start=True, stop=False)
                        nc.tensor.matmul(ps[:],
                                         ow_sb[:, 1, m * P:(m + 1) * P],
                                         attnT[1][:, ysl],
                                         start=False, stop=True)
                        nc.vector.tensor_copy(yt[:, m, :], ps[:])
                    for m in range(H // P):
                        nc.sync.dma_start(yT[m * P:(m + 1) * P, ysl],
                                          yt[:, m, :])

        if DEBUG:
            nc.sync.dma_start(dbg_attnT0, attnT[0][:])
            nc.sync.dma_start(dbg_attnT1, attnT[1][:])
            nc.sync.dma_start(dbg_q80, q8_sb[0][:])
            nc.sync.dma_start(dbg_q81, q8_sb[1][:])
            nc.sync.dma_start(dbg_khl0, khl_sb[0][:])
            nc.sync.dma_start(dbg_khl1, khl_sb[1][:])
            nc.sync.dma_start(dbg_v0, v_sb[0][:])
            nc.sync.dma_start(dbg_v800, v8_sb[0][0][:])
    nc.compile()
    return nc


def _alibi_slopes():
    n = NH // 2
    start = 2.0 ** (-(2.0 ** (-(np.log2(n) - 3.0))))
    s = np.array([start * start ** i for i in range(n)], dtype=np.float32)
    return np.concatenate([s, np.zeros(n, dtype=np.float32)])


def _hl8(a):
    """Split array into (hi, lo) fp8 e4m3 pair; hi + lo ~ a to ~12 bits."""
    hi = a.astype(NPF8)
    lo = (a - hi.astype(np.float32)).astype(NPF8)
    return hi, lo


def _pairs_P_NJ(hi, lo):
    """[L, L] hi/lo -> [P, NJ, 2, L] fp8 (partition-major j tiles)."""
    out = np.empty((P, NJ, 2, L), dtype=NPF8)
    out[:, :, 0, :] = hi.reshape(NJ, P, L).transpose(1, 0, 2)
    out[:, :, 1, :] = lo.reshape(NJ, P, L).transpose(1, 0, 2)
    return out


def _kp_pairs(a, cols):
    """[H, cols] fp8 -> [P, KP, 2, cols] (k-tile pair planes)."""
    return np.ascontiguousarray(
        a.reshape(KP, 2, P, cols).transpose(2, 0, 1, 3))


def _build_in_maps(x, adj, weights, in_bias, gamma, out_w, with_qk_bias):
    from concurrent.futures import ThreadPoolExecutor
    slopes = _alibi_slopes()
    ar = np.arange(L, dtype=np.float32)
    dist = -np.abs(ar[None, :] - ar[:, None])

    # dist = 16*hi + lo EXACTLY (lo is an integer in [-8, 8])
    dhi = (dist / 16.0).astype(NPF8)
    dlo = (dist - 16.0 * dhi.astype(np.float32)).astype(NPF8)
    distp = _pairs_P_NJ(dhi, dlo)

    def _adjp(b):
        adjT = np.ascontiguousarray(adj[b, 0].T)
        return _pairs_P_NJ(*_hl8(adjT))

    def _xp(b):
        xT = np.ascontiguousarray(x[b].T)
        xhi, xlo = _hl8(xT)
        return _kp_pairs(xhi, L), _kp_pairs(xlo, L)

    with ThreadPoolExecutor(max_workers=4) as ex:
        fut_adj = [ex.submit(_adjp, b) for b in range(B)]
        fut_x = [ex.submit(_xp, b) for b in range(B)]
        adjp_by_b = [f.result() for f in fut_adj]
        x_by_b = [f.result() for f in fut_x]

    identT = np.eye(P, dtype=NPBF16)
    idn_m = (-WS * np.eye(P)).astype(NPF8)
    ones = np.ones((1, IC), dtype=NPBF16)
    eye = np.eye(P, dtype=np.float32)

    in_maps = []
    for core in range(8):
        b, g = divmod(core, 4)
        heads = [8 + 2 * g, 9 + 2 * g, 4 + g, g]  # p0, p1, heavy, light
        qcols = np.concatenate([np.arange(192 * h, 192 * h + 64)
                                for h in heads])
        kcols = qcols + 64
        vcols = qcols + 128
        wqk = WS * weights[:, np.concatenate([qcols, kcols])]
        wqk_h, wqk_l = _hl8(wqk)
        wvm = np.zeros((H, 260), dtype=np.float32)
        for sl in range(4):
            wvm[:, 65 * sl:65 * sl + 64] = \
                weights[:, vcols[64 * sl:64 * sl + 64]]
        wv_h, wv_l = _hl8(WS * wvm)
        vbr = np.zeros((1, 260), dtype=np.float32)
        for sl in range(4):
            vbr[0, 65 * sl:65 * sl + 64] = \
                WS * in_bias[0, 0, vcols[64 * sl:64 * sl + 64]]
            vbr[0, 65 * sl + 64] = WS
        owm = np.ascontiguousarray(
            out_w[np.concatenate([np.arange(64 * h, 64 * h + 64)
                                  for h in heads]), :]).astype(NPBF16)
        idgm = np.zeros((P, 2, 512), dtype=NPF8)
        for sl in range(4):
            gi = (gamma[0, heads[sl], 0, 0] * eye).astype(NPF8)
            idgm[:, 0, sl * P:(sl + 1) * P] = gi
            idgm[:, 1, sl * P:(sl + 1) * P] = gi
        idslm = np.zeros((P, 2, 256), dtype=NPF8)
        idslm[:, 0, 0:P] = (16.0 * slopes[heads[2]] * eye).astype(NPF8)
        idslm[:, 1, 0:P] = (slopes[heads[2]] * eye).astype(NPF8)
        idslm[:, 0, P:2 * P] = (16.0 * slopes[heads[3]] * eye).astype(NPF8)
        idslm[:, 1, P:2 * P] = (slopes[heads[3]] * eye).astype(NPF8)
        m = {
            "xh": x_by_b[b][0], "xl": x_by_b[b][1],
            "wqh": _kp_pairs(wqk_h, 512), "wql": _kp_pairs(wqk_l, 512),
            "wvh": _kp_pairs(wv_h, 260), "wvl": _kp_pairs(wv_l, 260),
            "vb": vbr.astype(NPBF16), "ones": ones,
            "adjp": adjp_by_b[b], "distp": distp,
            "idg": idgm, "idsl": idslm, "identT": identT, "idn": idn_m,
            "ow": np.ascontiguousarray(
                owm.reshape(2, P, H).transpose(1, 0, 2)),
        }
        if with_qk_bias:
            m["qkb"] = (WS * in_bias[0, 0, np.concatenate([qcols, kcols])]
                        ).reshape(1, -1).astype(NPBF16)
        in_maps.append(m)
    return in_maps


def kernel(x, adj, weights, in_bias, out_w, out_bias, gamma):
    x = np.asarray(x, dtype=np.float32)
    adj = np.asarray(adj, dtype=np.float32)
    weights = np.asarray(weights, dtype=np.float32)
    in_bias = np.asarray(in_bias, dtype=np.float32)
    out_w = np.asarray(out_w, dtype=np.float32)
    out_bias = np.asarray(out_bias, dtype=np.float32)
    gamma = np.asarray(gamma, dtype=np.float32)

    with_qk_bias = bool(np.any(in_bias[0, 0, :]))
    key = f"nc_{with_qk_bias}"
    if key not in _cache:
        _cache[key] = _build_program(with_qk_bias)
    nc = _cache[key]

    in_maps = _build_in_maps(x, adj, weights, in_bias, gamma, out_w,
                             with_qk_bias)
    res = bass_utils.run_bass_kernel_spmd(nc, in_maps, core_ids=list(range(8)),
                                          **RUN_KWARGS)
    _cache["last_result"] = res

    out = np.empty((B, L, H), dtype=np.float32)
    for b in range(B):
        acc = res.results[4 * b]["yT"].astype(np.float32)
        for g in range(1, 4):
            acc += res.results[4 * b + g]["yT"]
        out[b] = acc.T + out_bias[0, 0][None, :]
    return out

